# revision 22
# baseline (speedup 1.0000x reference)
"""Trainium2 Bass kernel for BilinearPairedLayer.

Math (reference):
  h = relu(x @ W_lin + b_lin)                      # [B, N, 32]
  v = concat(shift(h,-1), h, shift(h,+1))          # [B, N, 96]
  out[b,i,j,o] = v[b,i] @ W_bil[o] @ v[b,j] + b_bil[o]   # [B, N, N, 8]

Kernel strategy (8 cores, shard over output column dim j):
  Each core owns a 128-wide j window. Contract W_bil with the j side first:
    u[b,j,o,h] = sum_g W_bil[o,h,g] v[b,j,g]       # per-core j slice
  then the main matmul per (b, i-chunk):
    out[i, (j,o)] = vT_aug[b].T @ u_aug[b]         # PSUM [128, 512]
  The (j,o) column order matches the DRAM layout, so output DMA is 512KB
  fully-contiguous blocks.

  Bias handling (keeps halo/edge zeros exact):
   - x is augmented host-side with a ones/indicator column; W_aug row 64 is
     b_lin, so h = relu(W_aug.T @ xT_aug) has the bias applied only on valid
     rows (indicator 0 on out-of-range halo rows -> h exactly 0).
   - vT_aug row 96 = 1.0 and u_aug row 96 = b_bil[o] adds the bilinear bias
     inside the main matmul.

  The per-core difference is carried purely in input data (xj = the core's
  j window of x, with 1-row halos), so all 8 cores run one identical NEFF.

Sync-wait budget (walrus: ONE wait slot per matmul / HWDGE DMA):
  - inputs arrive via single batched DMAs into persistent tiles (no slot
    reuse -> loads need no waits; one tick per tensor for PE to observe),
  - tiny "observer" matmuls absorb the ident/W DMA ticks into PE's clock,
  - every PSUM tile is read by DVE only (relu via tensor_scalar_max, all
    copies on nc.vector), so a matmul's remaining deps (operand producers +
    PSUM slot release) are all on the single DVE semaphore = one wait,
  - output DMAs go through gpsimd (SWDGE), which tolerates multiple waits.
"""

import os
import numpy as np
from contextlib import ExitStack

B, N, NIN, NH, NOUT = 4, 1024, 64, 32, 8
H = 3 * NH  # 96
NCORES = 8
NJ = N // NCORES  # 128 output columns per core
NA = NIN + 1  # 65: x augmented with ones/indicator column

_CACHE = {}


def _build_nc(use_f32r: bool):
    import concourse.bass as bass
    import concourse.tile as tile
    from concourse import bacc, mybir
    from concourse.masks import make_identity

    f32 = mybir.dt.float32
    mm_dt = mybir.dt.float32r if use_f32r else f32

    # Bacc (not raw Bass): its compile() runs generate_event_semaphores,
    # which splits multi-sync-waits into separate event instructions to
    # satisfy the one-wait-slot-per-instruction hardware constraint.
    nc = bacc.Bacc(
        "TRN2", target_bir_lowering=False, debug=False, num_devices=NCORES
    )

    x_d = nc.dram_tensor("x_aug", [B, N, NA], f32, kind="ExternalInput").ap()
    xj_d = nc.dram_tensor("xj", [B, 2, 128, NA], f32, kind="ExternalInput").ap()
    wa_d = nc.dram_tensor("W_aug", [NA, NH], f32, kind="ExternalInput").ap()
    # W_bilT[g, o, h] = W_bil[o, h, g]
    wb_d = nc.dram_tensor("W_bilT", [H, NOUT, H], f32, kind="ExternalInput").ap()
    bb_d = nc.dram_tensor("b_bil_t", [1, NJ, NOUT], f32, kind="ExternalInput").ap()
    # One output tensor per b, written by exactly one 4MB DMA (no WAW-chain
    # sync waits on DMA instructions, and max descriptor efficiency).
    out_d = [
        nc.dram_tensor(f"out_{b}", [N, NJ, NOUT], f32, kind="ExternalOutput").ap()
        for b in range(B)
    ]

    def mm(ap):
        return ap.bitcast(mm_dt) if use_f32r else ap

    with ExitStack() as ctx:
        tc = ctx.enter_context(tile.TileContext(nc))
        consts = ctx.enter_context(tc.tile_pool(name="consts", bufs=1))
        # 4 slots shared by xs, xjs and the 4 output staging tiles: the two
        # reused slots' previous readers are PE (transposes), which merges
        # into the drain's single PE wait.
        stage = ctx.enter_context(tc.tile_pool(name="stage", bufs=4))
        ps_tr = ctx.enter_context(tc.tile_pool(name="ps_tr", bufs=2, space="PSUM"))
        ps_u = ctx.enter_context(tc.tile_pool(name="ps_u", bufs=2, space="PSUM"))
        ps_m = ctx.enter_context(tc.tile_pool(name="ps_m", bufs=2, space="PSUM"))
        obsp = ctx.enter_context(tc.tile_pool(name="obsp", bufs=1, space="PSUM"))

        ident = consts.tile([128, 128], f32, tag="ident")
        make_identity(nc, ident)

        # Observer micro-matmuls: accumulate garbage into one never-read PSUM
        # tile. Each absorbs its operand's producer tick into PE's observed
        # clock so real matmuls don't need a second sync wait.
        obs = obsp.tile([1, 1], f32, tag="obs")
        _obs_first = [True]

        def observe(ap1):
            # All observers write the identical 4-byte PSUM region (keeps the
            # simulator's zero-region bookkeeping consistent); operand APs are
            # free-size-1 reads chosen to intersect each producer's region.
            assert ap1.free_size() == 1, ap1
            nc.tensor.matmul(
                obs[0:1, 0:1], lhsT=ap1, rhs=ap1, start=_obs_first[0], stop=False,
                skip_group_check=True,
            )
            _obs_first[0] = False

        observe(ident[0:1, 0:1])

        wa_sb = consts.tile([NA, NH], f32, tag="wa")
        nc.sync.dma_start(out=wa_sb, in_=wa_d)
        observe(wa_sb[0:1, 0:1])
        wb_sb = consts.tile([H, NOUT, H], f32, tag="wb")
        nc.sync.dma_start(out=wb_sb, in_=wb_d)
        observe(wb_sb[0:1, 0:1, 0:1])
        bb_sb = consts.tile([1, NJ, NOUT], f32, tag="bb")
        nc.sync.dma_start(out=bb_sb, in_=bb_d)

        # Single batched loads of x and xj (token-major, ones column included).
        xs = stage.tile([128, B * 8, NA], f32, tag="ot", name="xs")
        xjs = stage.tile([128, B * 2, NA], f32, tag="ot", name="xjs")
        with nc.allow_non_contiguous_dma(reason="batched row-tile load"):
            nc.sync.dma_start(
                out=xs, in_=x_d.rearrange("b (k p) a -> p (b k) a", p=128)
            )
            nc.sync.dma_start(
                out=xjs, in_=xj_d.rearrange("b k p a -> p (b k) a")
            )
        observe(xs[0:1, 0:1, 0:1])
        observe(xjs[0:1, 0:1, 0:1])

        # 1-elem DVE reads: absorb the xs/xjs load-DMA ticks into DVE's
        # clock (drains that reuse those staging slots then need only their
        # PE wait; partial transpose reads leave a residual WAW on the DMA).
        scrap_a = consts.tile([1, 1], f32, tag="scrap_a")
        scrap_b = consts.tile([1, 1], f32, tag="scrap_b")
        nc.vector.tensor_copy(scrap_a, xs[0:1, 0:1, 0:1])
        nc.vector.tensor_copy(scrap_b, xjs[0:1, 0:1, 0:1])

        xT = consts.tile([NA, B * N], f32, tag="xT")  # [65, 4096]
        xjT = consts.tile([NA, B * 256], f32, tag="xjT")  # [65, 1024]
        hT = consts.tile([128, N], f32, tag="hT")  # rows 32b+c, cols i
        hjT = consts.tile([128, 256], f32, tag="hjT")  # rows 32b+c, cols l
        vjt = consts.tile([H, B * NJ], f32, tag="vjt")  # [96, 512]

        # ---- transpose x (and xj) into feature-major layout ----
        for c in range(B * 8):
            pt = ps_tr.tile([NA, 128], f32, tag="ps")
            nc.tensor.matmul(pt, lhsT=xs[:, c, :], rhs=ident, start=True, stop=True)
            nc.vector.tensor_copy(xT[:, c * 128 : (c + 1) * 128], pt)
        for c in range(B * 2):
            pt = ps_tr.tile([NA, 128], f32, tag="ps")
            nc.tensor.matmul(pt, lhsT=xjs[:, c, :], rhs=ident, start=True, stop=True)
            nc.vector.tensor_copy(xjT[:, c * 128 : (c + 1) * 128], pt)

        # ---- h = relu(W_aug.T @ xT_aug), relu on DVE ----
        for b in range(B):
            for k in range(2):
                ph = ps_tr.tile([NH, 512], f32, tag="ps")
                nc.tensor.matmul(
                    ph,
                    lhsT=mm(wa_sb[:]),
                    rhs=mm(xT[:, b * 1024 + k * 512 : b * 1024 + (k + 1) * 512]),
                    start=True,
                    stop=True,
                )
                nc.vector.tensor_scalar_max(
                    hT[32 * b : 32 * b + 32, k * 512 : (k + 1) * 512], ph, 0.0
                )
            ph = ps_tr.tile([NH, 256], f32, tag="ps")
            nc.tensor.matmul(
                ph,
                lhsT=mm(wa_sb[:]),
                rhs=mm(xjT[:, b * 256 : (b + 1) * 256]),
                start=True,
                stop=True,
            )
            nc.vector.tensor_scalar_max(hjT[32 * b : 32 * b + 32, :], ph, 0.0)

        # ---- vT_aug per b: [97, N]; rows 0:32 h(i-1), 32:64 h(i), 64:96 h(i+1) ----
        vT = []
        for b in range(B):
            vT_b = consts.tile([H + 1, N], f32, tag=f"vT{b}", name=f"vT{b}")
            hb = hT[32 * b : 32 * b + 32, :]
            nc.vector.memset(vT_b[0:32, 0:1], 0.0)
            nc.vector.tensor_copy(vT_b[0:32, 1:N], hb[:, 0 : N - 1])
            nc.vector.tensor_copy(vT_b[32:64, :], hb)
            nc.vector.tensor_copy(vT_b[64:96, 0 : N - 1], hb[:, 1:N])
            nc.vector.memset(vT_b[64:96, N - 1 : N], 0.0)
            nc.vector.memset(vT_b[96:97, :], 1.0)
            vT.append(vT_b)

        # ---- v_jT packed over b: [96, 4*128]; local col l = jj + d ----
        for b in range(B):
            for d in range(3):
                nc.vector.tensor_copy(
                    vjt[32 * d : 32 * (d + 1), b * 128 : (b + 1) * 128],
                    hjT[32 * b : 32 * b + 32, d : d + 128],
                )

        # ---- u[b,j,o,h] = sum_g W_bil[o,h,g] v[b,j,g]; u_aug row 96 = b_bil ----
        # One packed tile for all b; each PSUM result is drained by a single
        # full-tile copy (partial reads leave a residual WAW writer-dep that
        # would add a second sync wait on the next matmul using the slot).
        u_all = consts.tile([H + 1, B, NJ, NOUT], f32, tag="u_all")
        for b in range(B):
            nc.vector.tensor_copy(u_all[96:97, b, :, :], bb_sb)
        for o in range(NOUT):
            pu = ps_u.tile([H, B * NJ], f32, tag="ps")
            nc.tensor.matmul(
                pu, lhsT=mm(wb_sb[:, o, :]), rhs=mm(vjt[:]), start=True, stop=True
            )
            nc.vector.tensor_copy(u_all[0:96, :, :, o], pu[:, :])

        # ---- main: out[i, (j,o)] = vT_aug.T @ u_aug, then one 4MB DMA per b ----
        for b in range(B):
            # Absorb this b's vT/u producer ticks into PE's observed clock so
            # each main matmul carries only its PSUM-slot-release wait.
            observe(vT[b][0:1, 0:1])
            observe(vT[b][0:1, 1:2])
            observe(vT[b][32:33, 0:1])
            observe(vT[b][64:97, 0:1])
            observe(vT[b][64:65, N - 1 : N])
            for o in range(NOUT):
                observe(u_all[0:1, b, 0:1, o : o + 1])
            observe(u_all[64:97, b, 0:1, 0:1])
            ot = stage.tile([128, 8, NJ * NOUT], f32, tag="ot", name=f"ot{b}")
            for ic in range(8):
                for jh in range(2):
                    pm = ps_m.tile([128, 512], f32, tag="ps")
                    nc.tensor.matmul(
                        pm,
                        lhsT=mm(vT[b][:, ic * 128 : (ic + 1) * 128]),
                        rhs=mm(u_all[:, b, jh * 64 : (jh + 1) * 64, :]),
                        start=True,
                        stop=True,
                    )
                    nc.vector.tensor_copy(
                        ot[:, ic, jh * 512 : (jh + 1) * 512], pm
                    )
            # SWDGE (gpsimd) DMA tolerates the multiple sync waits these
            # carry (HWDGE pseudo-DMAs have a single wait slot).
            nc.gpsimd.dma_start(
                out=out_d[b].rearrange("(ic p) j o -> p ic (j o)", p=128), in_=ot
            )

    nc.compile()
    return nc


def _prep_inputs(x, W_lin, b_lin, W_bil, b_bil):
    x = np.ascontiguousarray(x, dtype=np.float32)
    ones = np.ones((B, N, 1), dtype=np.float32)
    x_aug = np.concatenate([x, ones], axis=2)  # [B, N, 65]
    xpad = np.zeros((B, N + 2, NA), dtype=np.float32)
    xpad[:, 1 : N + 1] = x_aug  # rows 0 and N+1 are zero (indicator 0)

    W_aug = np.concatenate(
        [np.asarray(W_lin, np.float32), np.asarray(b_lin, np.float32)[None, :]],
        axis=0,
    )  # [65, 32]
    W_bilT = np.ascontiguousarray(
        np.asarray(W_bil, np.float32).transpose(2, 0, 1)
    )  # [g, o, h]
    b_bil_t = np.ascontiguousarray(
        np.tile(np.asarray(b_bil, np.float32)[None, :], (NJ, 1))[None]
    )  # [1, 128, 8]

    shared = {"x_aug": x_aug, "W_aug": W_aug, "W_bilT": W_bilT, "b_bil_t": b_bil_t}
    in_maps = []
    for c in range(NCORES):
        xj = np.zeros((B, 256, NA), dtype=np.float32)
        # local row l corresponds to global j = c*128 - 1 + l, for l in [0, 130)
        xj[:, :130] = xpad[:, c * NJ : c * NJ + 130]
        in_maps.append(dict(shared, xj=xj.reshape(B, 2, 128, NA)))
    return in_maps


def _run(inputs, trace=False, use_f32r=None):
    from concourse.bass_utils import run_bass_kernel_spmd

    if use_f32r is None:
        use_f32r = os.environ.get("KERNEL_F32R", "1") == "1"
    key = ("nc", bool(use_f32r))
    if key not in _CACHE:
        _CACHE[key] = _build_nc(use_f32r)
    nc = _CACHE[key]

    in_maps = _prep_inputs(
        inputs["x"], inputs["W_lin"], inputs["b_lin"], inputs["W_bil"], inputs["b_bil"]
    )
    res = run_bass_kernel_spmd(nc, in_maps, core_ids=list(range(NCORES)), trace=trace)
    out = np.empty((B, N, N, NOUT), dtype=np.float32)
    for c, r in enumerate(res.results):
        for b in range(B):
            out[b, :, c * NJ : (c + 1) * NJ, :] = r[f"out_{b}"]
    return out, res


def kernel(**inputs):
    out, _ = _run(inputs, trace=False)
    return out
